# revision 9
# baseline (speedup 1.0000x reference)
"""Per-channel affine (out = x * scale[c % 6] + shift[c % 6]) on a
(32768, 768) f32 tensor, data-parallel over 8 NeuronCores.

Each core gets a (4096, 768) row shard, viewed as [128 partitions x 24576
free] (each partition covers 32 contiguous rows; since 768 % 6 == 0 the
channel of an element is free_index % 6). The whole shard lives in one SBUF
tensor (96 KB/partition); the free dim is processed in chunks:

  SP  (HWDGE ring):  chunk loads, no waits, queued back-to-back
  DVE:               per chunk, one wait on the load sem, then 6 in-place
                     fused tensor_scalar ops (one per channel, stride-6 APs)
  ACT (HWDGE ring):  per chunk, one wait on the compute sem, then the store

Raw Bass blocks (not Tile) because this toolchain's walrus rejects any
instruction carrying more than one sync wait; explicit single-sem waits
keep every instruction at <= 1. The kernel is HBM-bandwidth-bound
(~25 MB traffic per core, ~70 us roofline at ~358 GB/s per core).
"""

from contextlib import ExitStack

import numpy as np

import concourse.bass as bass
import concourse.mybir as mybir
from concourse.bass_utils import run_bass_kernel_spmd

B, F = 32768, 768
N_CORES = 8
BS = B // N_CORES  # 4096 rows per core
P = 128
NF = (BS // P) * F  # 24576 free elements per partition
CHUNK = 3072  # 1.5 MB per [128, CHUNK] f32 chunk; divisible by 6
N_CHUNKS = NF // CHUNK

# Constants from the module (match reference.py's f32 rounding).
X_STD, Y_STD, Z_STD, L_STD, T_STD = 98.15, 98.15, 173.2, 69.28, 51.96
W_STD = 24.55
SCALE = [
    340.0 / X_STD, 340.0 / Y_STD, 600.0 / Z_STD,
    240.0 / L_STD, 144.0 / W_STD, 180.0 / T_STD,
]
SHIFT = [
    -170.0 / X_STD, -170.0 / Y_STD, -300.0 / Z_STD,
    (60.0 - 180.0) / L_STD, (6.0 - 36.66) / W_STD, -90.0 / T_STD,
]
SCALE = [float(np.float32(s)) for s in SCALE]
SHIFT = [float(np.float32(s)) for s in SHIFT]


def build_nc(repeat: int = 1) -> bass.Bass:
    """repeat > 1 builds a timing variant that streams the whole pipeline
    (load -> affine -> store) `repeat` times inside one NEFF, so two wall
    timings at different repeats isolate the per-iteration HW time. The
    graded kernel path uses repeat=1."""
    nc = bass.Bass()
    x = nc.declare_dram_parameter("x", [BS, F], mybir.dt.float32, isOutput=False)
    y = nc.declare_dram_parameter("y", [BS, F], mybir.dt.float32, isOutput=True)
    xv = x.rearrange("(p a) f -> p (a f)", p=P)
    yv = y.rearrange("(p a) f -> p (a f)", p=P)

    with (
        nc.sbuf_tensor([P, NF], mybir.dt.float32) as t,
        ExitStack() as es,
        nc.Block() as block,
    ):
        # One sem per input chunk: several loads are in flight at once, and
        # CoreSim's race detector rejects concurrent updates to one sem.
        # (Across repeats the sems are reused with higher thresholds, which
        # is HW-safe: HWDGE rings drain in FIFO order per issuing engine.)
        in_sems = [
            es.enter_context(nc.semaphore(f"in_sem{c}")) for c in range(N_CHUNKS)
        ]
        cmp_sem = es.enter_context(nc.semaphore("cmp_sem"))
        out_sems = [
            es.enter_context(nc.semaphore(f"out_sem{c}")) for c in range(N_CHUNKS)
        ]
        tg = t[:].rearrange("p (g c) -> p g c", c=6)

        @block.sync
        def _(sync):
            for r in range(repeat):
                for c in range(N_CHUNKS):
                    j0 = c * CHUNK
                    if r > 0:
                        # WAR: chunk c of repeat r-1 must be computed and
                        # stored before its SBUF region is overwritten.
                        sync.wait_ge(cmp_sem, N_CHUNKS * (r - 1) + c + 1)
                        sync.wait_ge(out_sems[c], 16 * r)
                    sync.dma_start(
                        out=t[:, j0 : j0 + CHUNK], in_=xv[:, j0 : j0 + CHUNK]
                    ).then_inc(in_sems[c], 16)

        @block.vector
        def _(vector):
            for r in range(repeat):
                for c in range(N_CHUNKS):
                    g0 = c * (CHUNK // 6)
                    vector.wait_ge(in_sems[c], 16 * (r + 1))
                    for k in range(6):
                        ins = vector.tensor_scalar(
                            out=tg[:, g0 : g0 + CHUNK // 6, k],
                            in0=tg[:, g0 : g0 + CHUNK // 6, k],
                            scalar1=SCALE[k],
                            scalar2=SHIFT[k],
                            op0=mybir.AluOpType.mult,
                            op1=mybir.AluOpType.add,
                        )
                        if k == 5:
                            ins.then_inc(cmp_sem, 1)

        @block.scalar
        def _(scalar):
            for r in range(repeat):
                for c in range(N_CHUNKS):
                    j0 = c * CHUNK
                    scalar.wait_ge(cmp_sem, N_CHUNKS * r + c + 1)
                    scalar.dma_start(
                        out=yv[:, j0 : j0 + CHUNK], in_=t[:, j0 : j0 + CHUNK]
                    ).then_inc(out_sems[c], 16)

    return nc


_nc_cache = None


def _get_nc() -> bass.Bass:
    global _nc_cache
    if _nc_cache is None:
        _nc_cache = build_nc()
    return _nc_cache


def run(x: np.ndarray, **spmd_kwargs):
    """Run the kernel; returns (full_output, BassKernelResults)."""
    nc = _get_nc()
    x = np.ascontiguousarray(np.asarray(x, dtype=np.float32))
    assert x.shape == (B, F), x.shape
    in_maps = [{"x": x[i * BS : (i + 1) * BS]} for i in range(N_CORES)]
    res = run_bass_kernel_spmd(nc, in_maps, list(range(N_CORES)), **spmd_kwargs)
    out = np.concatenate([r["y"] for r in res.results], axis=0)
    return out, res


def kernel(x: np.ndarray) -> np.ndarray:
    out, _ = run(x)
    return out


# revision 10
# speedup vs baseline: 3.6224x; 3.6224x over previous
"""Per-channel affine (out = x * scale[c % 6] + shift[c % 6]) on a
(32768, 768) f32 tensor, data-parallel over 8 NeuronCores.

Each core gets a (4096, 768) row shard, viewed as [128 partitions x 24576
free] (each partition covers 32 contiguous rows; since 768 % 6 == 0 the
channel of an element is free_index % 6). The whole shard lives in one SBUF
tensor (96 KB/partition); the free dim is processed in chunks:

  SP  (HWDGE ring):  chunk loads, no waits, queued back-to-back
  DVE:               per chunk, one wait on the load sem, then 6 in-place
                     fused tensor_scalar ops (one per channel, stride-6 APs)
  ACT (HWDGE ring):  per chunk, one wait on the compute sem, then the store

Raw Bass blocks (not Tile) because this toolchain's walrus rejects any
instruction carrying more than one sync wait; explicit single-sem waits
keep every instruction at <= 1. The kernel is HBM-bandwidth-bound
(~25 MB traffic per core, ~70 us roofline at ~358 GB/s per core).
"""

from contextlib import ExitStack

import numpy as np

import concourse.bass as bass
import concourse.mybir as mybir
from concourse.bass_utils import run_bass_kernel_spmd

B, F = 32768, 768
N_CORES = 8
BS = B // N_CORES  # 4096 rows per core
P = 128
NF = (BS // P) * F  # 24576 free elements per partition
CHUNK = 3072  # 1.5 MB per [128, CHUNK] f32 chunk; divisible by 6
N_CHUNKS = NF // CHUNK

# Constants from the module (match reference.py's f32 rounding).
X_STD, Y_STD, Z_STD, L_STD, T_STD = 98.15, 98.15, 173.2, 69.28, 51.96
W_STD = 24.55
SCALE = [
    340.0 / X_STD, 340.0 / Y_STD, 600.0 / Z_STD,
    240.0 / L_STD, 144.0 / W_STD, 180.0 / T_STD,
]
SHIFT = [
    -170.0 / X_STD, -170.0 / Y_STD, -300.0 / Z_STD,
    (60.0 - 180.0) / L_STD, (6.0 - 36.66) / W_STD, -90.0 / T_STD,
]
SCALE = [float(np.float32(s)) for s in SCALE]
SHIFT = [float(np.float32(s)) for s in SHIFT]


def build_nc(repeat: int = 1) -> bass.Bass:
    """repeat > 1 builds a timing variant that streams the whole pipeline
    (load -> affine -> store) `repeat` times inside one NEFF, so two wall
    timings at different repeats isolate the per-iteration HW time. The
    graded kernel path uses repeat=1."""
    nc = bass.Bass()
    x = nc.declare_dram_parameter("x", [BS, F], mybir.dt.float32, isOutput=False)
    y = nc.declare_dram_parameter("y", [BS, F], mybir.dt.float32, isOutput=True)
    xv = x.rearrange("(p a) f -> p (a f)", p=P)
    yv = y.rearrange("(p a) f -> p (a f)", p=P)

    with (
        nc.sbuf_tensor([P, NF], mybir.dt.float32) as t,
        ExitStack() as es,
        nc.Block() as block,
    ):
        # One sem per input chunk: several loads are in flight at once, and
        # CoreSim's race detector rejects concurrent updates to one sem.
        # (Across repeats the sems are reused with higher thresholds, which
        # is HW-safe: HWDGE rings drain in FIFO order per issuing engine.)
        in_sems = [
            es.enter_context(nc.semaphore(f"in_sem{c}")) for c in range(N_CHUNKS)
        ]
        cmp_sem = es.enter_context(nc.semaphore("cmp_sem"))
        out_sems = [
            es.enter_context(nc.semaphore(f"out_sem{c}")) for c in range(N_CHUNKS)
        ]
        tg = t[:].rearrange("p (g c) -> p g c", c=6)

        # Phase-separated: the whole load stream runs direction-pure, then
        # the whole store stream. Mixed-direction DMA measured ~383 GB/s
        # per core vs ~630-680 single-direction, so keeping HBM traffic
        # direction-pure beats load/store interleaving. Compute trails the
        # load stream chunk-by-chunk on DVE, so the store phase starts as
        # soon as the last load lands (+ first chunk's compute, long done).

        @block.sync
        def _(sync):
            for r in range(repeat):
                if r > 0:
                    # WAR: repeat r-1's stores must finish before reloading.
                    sync.wait_ge(out_sems[N_CHUNKS - 1], 16 * r)
                for c in range(N_CHUNKS):
                    j0 = c * CHUNK
                    sync.dma_start(
                        out=t[:, j0 : j0 + CHUNK], in_=xv[:, j0 : j0 + CHUNK]
                    ).then_inc(in_sems[c], 16)

        @block.vector
        def _(vector):
            for r in range(repeat):
                for c in range(N_CHUNKS):
                    g0 = c * (CHUNK // 6)
                    vector.wait_ge(in_sems[c], 16 * (r + 1))
                    for k in range(6):
                        ins = vector.tensor_scalar(
                            out=tg[:, g0 : g0 + CHUNK // 6, k],
                            in0=tg[:, g0 : g0 + CHUNK // 6, k],
                            scalar1=SCALE[k],
                            scalar2=SHIFT[k],
                            op0=mybir.AluOpType.mult,
                            op1=mybir.AluOpType.add,
                        )
                        if k == 5:
                            ins.then_inc(cmp_sem, 1)

        @block.scalar
        def _(scalar):
            for r in range(repeat):
                # Phase separation: stores start only after every load of
                # this repeat has landed.
                scalar.wait_ge(in_sems[N_CHUNKS - 1], 16 * (r + 1))
                for c in range(N_CHUNKS):
                    j0 = c * CHUNK
                    scalar.wait_ge(cmp_sem, N_CHUNKS * r + c + 1)
                    scalar.dma_start(
                        out=yv[:, j0 : j0 + CHUNK], in_=t[:, j0 : j0 + CHUNK]
                    ).then_inc(out_sems[c], 16)

    return nc


_nc_cache = None


def _get_nc() -> bass.Bass:
    global _nc_cache
    if _nc_cache is None:
        _nc_cache = build_nc()
    return _nc_cache


def run(x: np.ndarray, **spmd_kwargs):
    """Run the kernel; returns (full_output, BassKernelResults)."""
    nc = _get_nc()
    x = np.ascontiguousarray(np.asarray(x, dtype=np.float32))
    assert x.shape == (B, F), x.shape
    in_maps = [{"x": x[i * BS : (i + 1) * BS]} for i in range(N_CORES)]
    res = run_bass_kernel_spmd(nc, in_maps, list(range(N_CORES)), **spmd_kwargs)
    out = np.concatenate([r["y"] for r in res.results], axis=0)
    return out, res


def kernel(x: np.ndarray) -> np.ndarray:
    out, _ = run(x)
    return out
